# revision 1
# baseline (speedup 1.0000x reference)
"""GRU cell on 8 Trainium2 NeuronCores — data-parallel over batch.

Math (per batch row):
    z = sigmoid([x, h] @ W_z + b_z)
    r = sigmoid([x, h] @ W_r + b_r)
    n = tanh(x @ W_n[:D] + (r * h) @ W_n[D:] + b_n)
    h' = (1 - z) * h + z * n = h + z * (n - h)

Distribution: batch 8192 is split 1024 rows per core; weights are
replicated. Everything on-device is computed in a transposed layout
[hidden, batch] so both matmul operands have the contraction dim on
SBUF partitions and no on-device transpose is needed:
    out.T[ho, b] = sum_k W[k, ho] * xh.T[k, b]
The host pre-transposes x/h (free) and transposes the result back.

Matmuls run in bf16 (fp32 PSUM accumulation): 1 cycle/row on the PE
vs 4 for fp32.
"""

import os
import sys
import types

import numpy as np

import concourse.bass as bass
import concourse.tile as tile
from concourse import bacc, mybir
from concourse._compat import with_exitstack
from concourse.bass_interp import get_hw_module
from concourse.bass_utils import run_bass_kernel_spmd

try:
    from ml_dtypes import bfloat16 as np_bf16
except ImportError:  # pragma: no cover
    import jax.numpy as jnp

    np_bf16 = jnp.bfloat16

N_CORES = 8
D = 2048  # input size
H = 2048  # hidden size
BATCH = 8192
BC = BATCH // N_CORES  # batch per core (1024)
K = D + H  # contraction dim (4096)
P = 128  # partitions
KT = K // P  # k-chunks (32)
DT = D // P  # k-chunks covering the x part (16)
JT = H // P  # hidden-out tiles (16)
NF = 512  # moving free dim per matmul (one PSUM bank of fp32)
NB = BC // NF  # batch blocks per core (2)

f32 = mybir.dt.float32
bf16 = mybir.dt.bfloat16


def _install_ntff_hook():
    """antenv.axon_hooks isn't injected in this image; shim it so
    run_bass_kernel_spmd(trace=True) can capture NTFF profiles."""
    if "antenv.axon_hooks" in sys.modules:
        return
    try:
        from trn_agent_boot.trn_boot import _ntff_profile_via_ctypes

        hook = _ntff_profile_via_ctypes("/opt/axon/libaxon_pjrt.so")
    except Exception:
        hook = None
    mod = types.ModuleType("antenv.axon_hooks")
    mod.get_axon_ntff_profile_hook = lambda: hook
    mod.set_axon_ntff_profile_hook = lambda h: None
    sys.modules["antenv.axon_hooks"] = mod


@with_exitstack
def _gru_tile_kernel(ctx, tc, xh, wz, wr, wn, bz, br, bn, out):
    nc = tc.nc
    Sigmoid = mybir.ActivationFunctionType.Sigmoid
    Tanh = mybir.ActivationFunctionType.Tanh

    const_pool = ctx.enter_context(tc.tile_pool(name="const", bufs=1))
    xh_pool = ctx.enter_context(tc.tile_pool(name="xhp", bufs=1))
    rh_pool = ctx.enter_context(tc.tile_pool(name="rhp", bufs=1))
    w_pool = ctx.enter_context(tc.tile_pool(name="wp", bufs=4))
    act_pool = ctx.enter_context(tc.tile_pool(name="actp", bufs=3))
    out_pool = ctx.enter_context(tc.tile_pool(name="outp", bufs=3))
    psum_pool = ctx.enter_context(tc.tile_pool(name="psp", bufs=8, space="PSUM"))

    # Biases as [128, JT]: column j holds bias[j*128 : (j+1)*128].
    bias_sb = {}
    for name, ap in (("z", bz), ("r", br), ("n", bn)):
        t = const_pool.tile([P, JT], f32, name=f"bias_{name}")
        nc.sync.dma_start(t[:], ap.rearrange("(j p) -> p j", p=P))
        bias_sb[name] = t

    # Resident activations: xh.T as KT chunks of [128, BC] (bf16).
    xh_sb = xh_pool.tile([P, KT * BC], bf16, name="xh_sb")
    for t in range(KT):
        nc.sync.dma_start(xh_sb[:, t * BC : (t + 1) * BC], xh[t * P : (t + 1) * P, :])

    def xh_chunk(t, b_i):
        return xh_sb[:, t * BC + b_i * NF : t * BC + (b_i + 1) * NF]

    def h_chunk(j):  # h_prev.T chunk j, [128, BC]
        return xh_sb[:, (DT + j) * BC : (DT + j + 1) * BC]

    # r * h_prev (transposed), filled during the r phase.
    rh_sb = rh_pool.tile([P, JT * BC], bf16, name="rh_sb")

    def load_w_cols(w_ap, j, name):
        """[128, KT*128] tile: cols t*128.. hold W[t*128+p, j*128+..]."""
        wt = w_pool.tile([P, KT * P], bf16, tag="w", name=name)
        nc.sync.dma_start(
            wt[:].rearrange("p (t h) -> p t h", h=P),
            w_ap.rearrange("(t p) h -> p t h", p=P)[:, :, j * P : (j + 1) * P],
        )
        return wt

    def accumulate(ps, w_tile, rhs_of_t):
        for t in range(KT):
            lhsT = w_tile[:, t * P : (t + 1) * P]
            for b_i in range(NB):
                nc.tensor.matmul(
                    ps[b_i][:],
                    lhsT,
                    rhs_of_t(t, b_i),
                    start=(t == 0),
                    stop=(t == KT - 1),
                )

    # ---- phase R: r gate, then rh = r * h_prev ----
    for j in range(JT):
        wr_j = load_w_cols(wr, j, "wr_j")
        ps = [psum_pool.tile([P, NF], f32, tag="ps", name="ps_r") for _ in range(NB)]
        accumulate(ps, wr_j, xh_chunk)
        r_j = act_pool.tile([P, BC], bf16, tag="r", name="r_j")
        for b_i in range(NB):
            nc.scalar.activation(
                r_j[:, b_i * NF : (b_i + 1) * NF],
                ps[b_i][:],
                Sigmoid,
                bias=bias_sb["r"][:, j : j + 1],
            )
        nc.vector.tensor_mul(rh_sb[:, j * BC : (j + 1) * BC], r_j[:], h_chunk(j))

    # ---- phase NZ: z and n gates + combine ----
    for j in range(JT):
        wz_j = load_w_cols(wz, j, "wz_j")
        wn_j = load_w_cols(wn, j, "wn_j")
        psz = [psum_pool.tile([P, NF], f32, tag="ps", name="ps_z") for _ in range(NB)]
        psn = [psum_pool.tile([P, NF], f32, tag="ps", name="ps_n") for _ in range(NB)]
        accumulate(psz, wz_j, xh_chunk)

        def n_rhs(t, b_i):
            if t < DT:
                return xh_chunk(t, b_i)
            tt = t - DT
            return rh_sb[:, tt * BC + b_i * NF : tt * BC + (b_i + 1) * NF]

        accumulate(psn, wn_j, n_rhs)

        z_j = act_pool.tile([P, BC], f32, tag="z", name="z_j")
        n_j = act_pool.tile([P, BC], f32, tag="n", name="n_j")
        for b_i in range(NB):
            sl = slice(b_i * NF, (b_i + 1) * NF)
            nc.scalar.activation(
                z_j[:, sl], psz[b_i][:], Sigmoid, bias=bias_sb["z"][:, j : j + 1]
            )
            nc.scalar.activation(
                n_j[:, sl], psn[b_i][:], Tanh, bias=bias_sb["n"][:, j : j + 1]
            )

        # h' = h + z * (n - h)
        d_j = act_pool.tile([P, BC], f32, tag="d", name="d_j")
        nc.vector.tensor_sub(d_j[:], n_j[:], h_chunk(j))
        zd_j = act_pool.tile([P, BC], f32, tag="zd", name="zd_j")
        nc.vector.tensor_mul(zd_j[:], z_j[:], d_j[:])
        o_j = out_pool.tile([P, BC], f32, name="o_j")
        nc.vector.tensor_add(o_j[:], zd_j[:], h_chunk(j))
        nc.sync.dma_start(out[j * P : (j + 1) * P, :], o_j[:])


_CACHED = None


def _build():
    global _CACHED
    if _CACHED is not None:
        return _CACHED
    nc = bacc.Bacc(
        "TRN2", target_bir_lowering=False, debug=False, enable_asserts=False
    )
    xh = nc.dram_tensor("xh", [K, BC], bf16, kind="ExternalInput").ap()
    wz = nc.dram_tensor("wz", [K, H], bf16, kind="ExternalInput").ap()
    wr = nc.dram_tensor("wr", [K, H], bf16, kind="ExternalInput").ap()
    wn = nc.dram_tensor("wn", [K, H], bf16, kind="ExternalInput").ap()
    bz = nc.dram_tensor("bz", [H], f32, kind="ExternalInput").ap()
    br = nc.dram_tensor("br", [H], f32, kind="ExternalInput").ap()
    bn = nc.dram_tensor("bn", [H], f32, kind="ExternalInput").ap()
    out = nc.dram_tensor("out", [H, BC], f32, kind="ExternalOutput").ap()

    with tile.TileContext(nc) as tc:
        _gru_tile_kernel(tc, xh, wz, wr, wn, bz, br, bn, out)
    nc.compile()
    nc.m = get_hw_module(nc.m)
    _CACHED = nc
    return nc


def _make_in_maps(x, h_prev, W_z, b_z, W_r, b_r, W_n, b_n):
    wz16 = np.ascontiguousarray(W_z.astype(np_bf16))
    wr16 = np.ascontiguousarray(W_r.astype(np_bf16))
    wn16 = np.ascontiguousarray(W_n.astype(np_bf16))
    bz32 = np.ascontiguousarray(b_z.astype(np.float32))
    br32 = np.ascontiguousarray(b_r.astype(np.float32))
    bn32 = np.ascontiguousarray(b_n.astype(np.float32))
    in_maps = []
    for i in range(N_CORES):
        sl = slice(i * BC, (i + 1) * BC)
        xh_i = np.concatenate([x[sl].T, h_prev[sl].T], axis=0)
        in_maps.append(
            {
                "xh": np.ascontiguousarray(xh_i.astype(np_bf16)),
                "wz": wz16,
                "wr": wr16,
                "wn": wn16,
                "bz": bz32,
                "br": br32,
                "bn": bn32,
            }
        )
    return in_maps


LAST_RESULT = None


def kernel(x, h_prev, W_z, b_z, W_r, b_r, W_n, b_n):
    global LAST_RESULT
    trace = bool(os.environ.get("GRU_TRACE"))
    if trace:
        _install_ntff_hook()
    nc = _build()
    in_maps = _make_in_maps(x, h_prev, W_z, b_z, W_r, b_r, W_n, b_n)
    res = run_bass_kernel_spmd(
        nc, in_maps, core_ids=list(range(N_CORES)), trace=trace
    )
    LAST_RESULT = res
    outs = [res.results[i]["out"].T for i in range(N_CORES)]
    return np.ascontiguousarray(np.concatenate(outs, axis=0).astype(np.float32))

